# revision 36
# baseline (speedup 1.0000x reference)
"""BitLinear 1.58-bit (nn_BitLinear158) Trainium2 kernel, 8-core tensor-parallel.

Math (must match reference):
  gamma_x = max(max|x|, eps); s = 128/gamma_x; xq = clip(round(x*s), -128, 127)
  gamma_w = max(mean|w|, eps); wq = clip(round(w/gamma_w), -1, 1)  (ternary)
  out = (xq @ wq.T) * (gamma_w / s) + bias

Key facts exploited:
  - xq in [-128,127] and wq in {-1,0,1} are exact in bf16; products and all
    PSUM partial sums are integers < 2^20, exact in fp32 => the GEMM runs at
    full bf16 PE rate and is bit-identical to the fp32 reference einsum.
  - wq = 1[w > 0.5*gamma_w] - 1[w < -0.5*gamma_w] (no division / round):
    round-half-even of w/gamma at +-0.5 and the clip at +-1.5 make the single
    threshold exact.
  - round-half-even via the fp32 magic constant 1.5*2^23 (valid for |v|<=2^22).

Sharding: weight/bias split over out_features (16384 -> 8 x 2048), x
replicated; gamma_w needs an AllReduce of per-shard |w| sums.
"""

from contextlib import ExitStack

import numpy as np

import concourse.bass as bass
import concourse.mybir as mybir
import concourse.tile as tile
from concourse import bass_utils
from concourse.masks import make_identity
from concourse.vector_clock import ScopedClock

# ---------------------------------------------------------------------------
# Workaround: this walrus build rejects instructions carrying >1-2 sync wait
# commands. Tile's tail drain (emitted after tile_legalize) aggregates one
# wait per outstanding proc onto a single InstDrain and so escapes the
# wait-count legalization. Redistribute its waits across a chain of NO-queue
# nops (same sequencer => program order preserves the barrier semantics).
# ---------------------------------------------------------------------------
_MAX_WAITS = 1


def _patched_drain_and_barrier(self, tick_clock, wait_clock):
    nc = self.nc
    probe = nc.sync.nop()
    wait_clock.add_sem_waits(probe.ins, ScopedClock({None: tick_clock.global_clock}))
    si = probe.ins.sync_info
    waits = list(si.on_wait) if si is not None and si.on_wait else []
    ups = list(si.on_update) if si is not None and si.on_update else []
    probe.ins.sync_info = mybir.SyncInfo(on_wait=waits[:_MAX_WAITS], on_update=ups)
    rest = waits[_MAX_WAITS:]
    while rest:
        n2 = nc.sync.nop()
        n2.ins.sync_info = mybir.SyncInfo(on_wait=rest[:_MAX_WAITS], on_update=[])
        rest = rest[_MAX_WAITS:]

    nc.sync.drain()

    nc.all_engine_barrier()
    assert self.sems is not None
    popped = nc._tile_sem_poison_stack.pop()
    assert popped is self._sem_poison
    nc.clear_and_free_semaphores(list(self.sems.allocated().values()))
    nc.all_engine_barrier()


tile.TileContext._drain_and_barrier = _patched_drain_and_barrier

_nop_counter = [0]


def _legalize_waits(nc):
    """Split >_MAX_WAITS sync waits per instruction onto same-engine nops
    inserted immediately before (per-engine program order => semantics kept)."""
    for f in nc.m.functions:
        for blk in f.blocks:
            out = []
            changed = False
            for inst in blk.instructions:
                si = getattr(inst, "sync_info", None)
                waits = list(si.on_wait) if si is not None and si.on_wait else []
                if len(waits) > _MAX_WAITS and inst.engine != mybir.EngineType.Unassigned:
                    while len(waits) > _MAX_WAITS:
                        chunk, waits = waits[:_MAX_WAITS], waits[_MAX_WAITS:]
                        _nop_counter[0] += 1
                        out.append(mybir.InstNoOp(
                            name=f"waitnop-{_nop_counter[0]}",
                            engine=inst.engine, ins=[], outs=[],
                            sync_info=mybir.SyncInfo(on_wait=chunk, on_update=[]),
                        ))
                    inst.sync_info = mybir.SyncInfo(
                        on_wait=waits,
                        on_update=list(si.on_update) if si.on_update else [])
                    changed = True
                out.append(inst)
            if changed:
                blk.instructions = out


# ---------------------------------------------------------------------------

N_CORES = 8
B, S, IN_F, OUT_F = 4, 2048, 4096, 16384
M = B * S                    # 8192 rows of x
N_SH = OUT_F // N_CORES      # 2048 output features per core
KT = IN_F // 128             # 32 k-tiles
MT = M // 128                # 64 m-tiles
NCH = N_SH // 512            # 4 psum column chunks
WT = N_SH // 128             # 16 weight row-tiles per core
EPS = 1e-5
MAGIC = 12582912.0           # 1.5 * 2^23: fp32 round-to-nearest-even trick
F32 = mybir.dt.float32
BF16 = mybir.dt.bfloat16

_CACHE = {}


M_SL = M // N_CORES  # per-core slice of x for the pass-1 max (1024 rows)


def _build(collective=True):
    nc = bass.Bass("TRN2", target_bir_lowering=False, debug=False,
                   num_devices=N_CORES if collective else 1)
    x_ap = nc.dram_tensor("x", [M, IN_F], F32, kind="ExternalInput").ap()
    x1_ap = nc.dram_tensor("x1", [M_SL, IN_F], F32, kind="ExternalInput").ap()
    w_ap = nc.dram_tensor("w", [N_SH, IN_F], F32, kind="ExternalInput").ap()
    b_ap = nc.dram_tensor("b", [1, N_SH], F32, kind="ExternalInput").ap()
    o_ap = nc.dram_tensor("o", [M, N_SH], F32, kind="ExternalOutput").ap()

    with tile.TileContext(nc) as tc:
        with ExitStack() as stack:
            _body(nc, tc, stack, x_ap, x1_ap, w_ap, b_ap, o_ap,
                  collective=collective)
    _legalize_waits(nc)
    return nc


def _body(nc, tc, stack, x_ap, x1_ap, w_ap, b_ap, o_ap, collective=True):
    def pool(name, bufs, space="SBUF"):
        return stack.enter_context(
            tc.tile_pool(name=name, bufs=bufs, space=space))

    # --- persistent SBUF ---
    wq_pool = pool("wq", 1)
    # wqT layout: [128 k-part, KT * N_SH] bf16, k-tile major
    wqT = wq_pool.tile([128, KT * N_SH], BF16, name="wqT", tag="wqT")
    const_pool = pool("const", 1)
    ident_bf = const_pool.tile([128, 128], BF16, name="ident_bf", tag="ibf")
    ident_f32 = const_pool.tile([128, 128], F32, name="ident_f32", tag="if32")
    ones_row = const_pool.tile([1, 128], F32, name="ones_row", tag="ones")
    bias_rep = const_pool.tile([128, N_SH], F32, name="bias_rep", tag="brep")
    scal128 = const_pool.tile([128, 4], F32, name="scal128", tag="scal128")
    magic128 = const_pool.tile([128, 1], F32, name="magic128", tag="magic")
    stats_pool = pool("stats", 1)
    wsums = stats_pool.tile([128, WT * 2], F32, name="wsums", tag="wsums")
    xmaxs = stats_pool.tile([128, MT * 2], F32, name="xmaxs", tag="xmaxs")
    stats2 = stats_pool.tile([128, 2], F32, name="stats2", tag="stats2")
    statsT_w = stats_pool.tile([1, 128], F32, name="statsT_w", tag="statsTw")
    statsT_x = stats_pool.tile([1, 128], F32, name="statsT_x", tag="statsTx")
    sc = stats_pool.tile([1, 8], F32, name="sc", tag="sc")

    # --- rotating SBUF ---
    io_pool = pool("io", 3)          # [128, 2048] f32 halves of x / w rows
    xq_pool = pool("xq", 1)          # [128, 4096] bf16 quantized row-tile
    xqT_pool = pool("xqT", 1)        # [128, 4096] bf16 transposed row-tile
    out_pool = pool("out", 1)        # [128, 2048] f32 staging
    bch_pool = pool("bch", 1)        # [1, 512] f32 bias chunks

    make_identity(nc, ident_bf[:])
    make_identity(nc, ident_f32[:])
    nc.gpsimd.memset(ones_row[:], 1.0)
    nc.gpsimd.memset(magic128[:], MAGIC)

    # PSUM pools: prep (2 banks) + pt (3) live together; po (5) opens after
    # prep closes => never more than 8 banks. pt opened first (stack order:
    # prep must close while pt stays open).
    pt_pool = tc.tile_pool(name="pt", bufs=3, space="PSUM")
    pt = pt_pool.__enter__()
    psum_prep = tc.tile_pool(name="psum_prep", bufs=2, space="PSUM")
    pp = psum_prep.__enter__()

    # ---------------- pass 1: |w| row sums + sliced max|x| ----------------
    # x responsibility for the global max is M-sharded across cores (each
    # core scans 1/8 of x = its x1 input); an AllReduce(max) recovers the
    # exact global max (max is exact under any order).
    for j in range(WT * 2):
        w_h = io_pool.tile([128, 2048], F32, name=f"wh_{j}", tag="io")
        nc.sync.dma_start(w_h[:], w_ap[(j // 2) * 128:(j // 2 + 1) * 128,
                                       (j % 2) * 2048:(j % 2 + 1) * 2048])
        nc.vector.tensor_reduce(wsums[:, j:j + 1], w_h[:],
                                axis=mybir.AxisListType.X,
                                op=mybir.AluOpType.add,
                                apply_absolute_value=True)
    nc.vector.tensor_reduce(stats2[:, 0:1], wsums[:],
                            axis=mybir.AxisListType.X, op=mybir.AluOpType.add)

    NX1 = (M_SL // 128) * 2  # 16 half-tiles of the x slice
    for j in range(NX1):
        x_h = io_pool.tile([128, 2048], F32, name=f"xh1_{j}", tag="io")
        nc.sync.dma_start(x_h[:], x1_ap[(j // 2) * 128:(j // 2 + 1) * 128,
                                        (j % 2) * 2048:(j % 2 + 1) * 2048])
        nc.vector.tensor_reduce(xmaxs[:, j:j + 1], x_h[:],
                                axis=mybir.AxisListType.X,
                                op=mybir.AluOpType.max,
                                apply_absolute_value=True)
    nc.vector.tensor_reduce(stats2[:, 1:2], xmaxs[:, 0:NX1],
                            axis=mybir.AxisListType.X, op=mybir.AluOpType.max)

    # cross-partition reductions via PE transpose
    st_ps_w = pp.tile([1, 128], F32, name="st_ps_w", tag="prep")
    nc.tensor.transpose(st_ps_w[:], stats2[:, 0:1], ident_f32[:])
    nc.vector.tensor_copy(statsT_w[:], st_ps_w[:])
    nc.vector.tensor_reduce(sc[0:1, 4:5], statsT_w[:],
                            axis=mybir.AxisListType.X, op=mybir.AluOpType.add)
    st_ps_x = pp.tile([1, 128], F32, name="st_ps_x", tag="prep")
    nc.tensor.transpose(st_ps_x[:], stats2[:, 1:2], ident_f32[:])
    nc.vector.tensor_copy(statsT_x[:], st_ps_x[:])
    nc.vector.tensor_reduce(sc[0:1, 5:6], statsT_x[:],
                            axis=mybir.AxisListType.X, op=mybir.AluOpType.max)

    if collective:
        dram_pool = pool("dram", 1, space="DRAM")
        cc_in = dram_pool.tile([1, 2], F32, name="cc_in", tag="cc_in")
        cc_out_s = dram_pool.tile([1, 1], F32, name="cc_out_s", tag="cc_out_s",
                                  addr_space="Shared")
        cc_out_m = dram_pool.tile([1, 1], F32, name="cc_out_m", tag="cc_out_m",
                                  addr_space="Shared")
        nc.gpsimd.dma_start(cc_in[:], sc[0:1, 4:6])
        nc.gpsimd.collective_compute(
            "AllReduce", mybir.AluOpType.max,
            replica_groups=[list(range(N_CORES))],
            ins=[cc_in[0:1, 1:2].opt()], outs=[cc_out_m.opt()],
        )
        nc.gpsimd.collective_compute(
            "AllReduce", mybir.AluOpType.add,
            replica_groups=[list(range(N_CORES))],
            ins=[cc_in[0:1, 0:1].opt()], outs=[cc_out_s.opt()],
        )
        nc.gpsimd.dma_start(sc[0:1, 6:7], cc_out_s[:])
        nc.gpsimd.dma_start(sc[0:1, 5:6], cc_out_m[:])
        wsum_all = sc[0:1, 6:7]
        inv_cnt = 1.0 / (OUT_F * IN_F)
    else:  # single-core sim variant: local stats stand in for global ones
        wsum_all = sc[0:1, 4:5]
        inv_cnt = 1.0 / (N_SH * IN_F)

    # gamma_w = max(sum/count, eps)  -> sc[0,7]
    nc.vector.tensor_scalar(sc[0:1, 7:8], wsum_all,
                            inv_cnt, EPS,
                            op0=mybir.AluOpType.mult, op1=mybir.AluOpType.max)
    # thr = 0.5*gamma_w -> sc[0,2]; nthr -> sc[0,3]
    nc.vector.tensor_scalar(sc[0:1, 2:3], sc[0:1, 7:8], 0.5, None,
                            op0=mybir.AluOpType.mult)
    nc.vector.tensor_scalar(sc[0:1, 3:4], sc[0:1, 7:8], -0.5, None,
                            op0=mybir.AluOpType.mult)
    scw_ps = pp.tile([128, 2], F32, name="scw_ps", tag="prep")
    nc.tensor.matmul(scw_ps[:], ones_row[:], sc[0:1, 2:4], start=True, stop=True)
    nc.vector.tensor_copy(scal128[:, 2:4], scw_ps[:])
    thr128 = scal128[:, 2:3]
    nthr128 = scal128[:, 3:4]

    # -------- quantize + transpose the weight shard --------
    for r in range(WT):
        wq_t = xq_pool.tile([128, IN_F], BF16, name=f"wqt_{r}", tag="xq")
        for h in range(2):
            w_h = io_pool.tile([128, 2048], F32, name=f"wh2_{r}_{h}", tag="io")
            nc.sync.dma_start(w_h[:], w_ap[r * 128:(r + 1) * 128,
                                           h * 2048:(h + 1) * 2048])
            neg = out_pool.tile([128, 2048], F32, name=f"neg_{r}_{h}", tag="out")
            nc.gpsimd.tensor_scalar(neg[:], w_h[:], nthr128, None,
                                    op0=mybir.AluOpType.is_lt)
            # wq = (w > thr) - (w < -thr)   in {-1, 0, 1}, bf16
            nc.vector.scalar_tensor_tensor(
                wq_t[:, h * 2048:(h + 1) * 2048], w_h[:], thr128, neg[:],
                op0=mybir.AluOpType.is_gt, op1=mybir.AluOpType.subtract)
        for k in range(KT):
            ptt = pt.tile([128, 128], BF16, name=f"wpt_{r}_{k}", tag="pt")
            nc.tensor.transpose(ptt[:], wq_t[:, k * 128:(k + 1) * 128],
                                ident_bf[:])
            nc.any.tensor_copy(wqT[:, k * N_SH + r * 128: k * N_SH + (r + 1) * 128],
                               ptt[:])

    # gamma_x = max(global max, eps) in place of sc[0,5]
    nc.vector.tensor_scalar(sc[0:1, 5:6], sc[0:1, 5:6], EPS, None,
                            op0=mybir.AluOpType.max)
    # scale_x = 128 * (1/gamma_x) -> sc[0,0]
    nc.vector.reciprocal(sc[0:1, 0:1], sc[0:1, 5:6])
    nc.vector.tensor_scalar(sc[0:1, 0:1], sc[0:1, 0:1], 128.0, None,
                            op0=mybir.AluOpType.mult)
    # r = gamma_w * gamma_x / 128 -> sc[0,1]
    nc.vector.tensor_scalar(sc[0:1, 1:2], sc[0:1, 5:6], 1.0 / 128.0, None,
                            op0=mybir.AluOpType.mult)
    nc.vector.tensor_mul(sc[0:1, 1:2], sc[0:1, 1:2], sc[0:1, 7:8])
    scx_ps = pp.tile([128, 2], F32, name="scx_ps", tag="prep")
    nc.tensor.matmul(scx_ps[:], ones_row[:], sc[0:1, 0:2], start=True, stop=True)
    nc.vector.tensor_copy(scal128[:, 0:2], scx_ps[:])
    scale128 = scal128[:, 0:1]
    r128 = scal128[:, 1:2]

    # bias broadcast to 128 partitions
    for n in range(NCH):
        bch = bch_pool.tile([1, 512], F32, name=f"bch_{n}", tag="bch")
        nc.sync.dma_start(bch[:], b_ap[0:1, n * 512:(n + 1) * 512])
        b_ps = pp.tile([128, 512], F32, name=f"b_ps_{n}", tag="prep")
        nc.tensor.matmul(b_ps[:], ones_row[:], bch[:], start=True, stop=True)
        nc.vector.tensor_copy(bias_rep[:, n * 512:(n + 1) * 512], b_ps[:])
    psum_prep.__exit__(None, None, None)

    po_pool = tc.tile_pool(name="po", bufs=5, space="PSUM")
    po = po_pool.__enter__()

    # ---------------- main loop over m-tiles ----------------
    for i in range(MT):
        xq_t = xq_pool.tile([128, IN_F], BF16, name=f"xq_{i}", tag="xq")
        for h in range(2):
            x_h = io_pool.tile([128, 2048], F32, name=f"xh2_{i}_{h}", tag="io")
            nc.sync.dma_start(x_h[:], x_ap[i * 128:(i + 1) * 128,
                                           h * 2048:(h + 1) * 2048])
            # xs = round_to_int(x*s), in place: magic-add rounds half-to-even
            nc.scalar.activation(x_h[:], x_h[:],
                                 mybir.ActivationFunctionType.Identity,
                                 bias=magic128[:], scale=scale128)
            # xq = min(xs - magic, 127) -> bf16 (>= -128 by construction)
            nc.vector.tensor_scalar(xq_t[:, h * 2048:(h + 1) * 2048], x_h[:],
                                    MAGIC, 127.0,
                                    op0=mybir.AluOpType.subtract,
                                    op1=mybir.AluOpType.min)

        xqT_t = xqT_pool.tile([128, IN_F], BF16, name=f"xqT_{i}", tag="xqT")
        pous = [po.tile([128, 512], F32, name=f"po_{i}_{n}", tag="po")
                for n in range(NCH)]

        def transpose_k(k):
            ptt = pt.tile([128, 128], BF16, name=f"xpt_{i}_{k}", tag="pt")
            nc.tensor.transpose(ptt[:], xq_t[:, k * 128:(k + 1) * 128],
                                ident_bf[:])
            nc.any.tensor_copy(xqT_t[:, k * 128:(k + 1) * 128], ptt[:])

        # software-pipelined: T(k) runs on PE between MM(k-1) bursts
        transpose_k(0)
        transpose_k(1)
        for k in range(KT):
            for n in range(NCH):
                nc.tensor.matmul(
                    pous[n][:],
                    xqT_t[:, k * 128:(k + 1) * 128],
                    wqT[:, k * N_SH + n * 512: k * N_SH + (n + 1) * 512],
                    start=(k == 0), stop=(k == KT - 1))
            if k + 2 < KT:
                transpose_k(k + 2)

        o_t = out_pool.tile([128, N_SH], F32, name=f"ot_{i}", tag="out")
        for n in range(NCH):
            # out = psum * r + bias
            nc.vector.scalar_tensor_tensor(
                o_t[:, n * 512:(n + 1) * 512], pous[n][:], r128,
                bias_rep[:, n * 512:(n + 1) * 512],
                op0=mybir.AluOpType.mult, op1=mybir.AluOpType.add)
        nc.sync.dma_start(o_ap[i * 128:(i + 1) * 128, :], o_t[:])

    po_pool.__exit__(None, None, None)
    pt_pool.__exit__(None, None, None)


def kernel(**inputs):
    x = np.ascontiguousarray(inputs["input"], dtype=np.float32).reshape(M, IN_F)
    w = np.ascontiguousarray(inputs["weight"], dtype=np.float32)
    b = np.ascontiguousarray(inputs["bias"], dtype=np.float32)

    if "nc" not in _CACHE:
        _CACHE["nc"] = _build()
    nc = _CACHE["nc"]

    in_maps = []
    for c in range(N_CORES):
        in_maps.append({
            "x": x,
            "x1": x[c * M_SL:(c + 1) * M_SL],
            "w": w[c * N_SH:(c + 1) * N_SH],
            "b": b[c * N_SH:(c + 1) * N_SH].reshape(1, N_SH),
        })
    res = bass_utils.run_bass_kernel_spmd(nc, in_maps,
                                          core_ids=list(range(N_CORES)))
    _CACHE["last_results"] = res
    out = np.concatenate([r["o"] for r in res.results], axis=1)
    return out.reshape(B, S, OUT_F)
